# revision 6
# baseline (speedup 1.0000x reference)
"""FFT-Conv2d as direct valid cross-correlation on Trainium2 (Bass/Tile).

Math: the reference's rfft2/einsum/irfft2 pipeline is exactly a *valid*
2-D cross-correlation plus bias:

    out[b, d, i, j] = sum_{c,u,v} signal[b, c, i+u, j+v] * weight[d, c, u, v]
                      + bias[d]

with signal [16, 32, 256, 256], weight [32, 32, 31, 31] -> out [16, 32, 226, 226].

Device strategy (data-parallel, 2 batch images per NeuronCore x 8 cores):
  - Contraction dim (PE partition axis, K=128) packs 4 kernel rows x 32
    input channels.  The signal is replicated into SBUF 4x with row shifts
    of 0..3 so that one SBUF row slice provides all 128 contraction rows.
  - Output dim (PE partition axis of PSUM, M=128) packs 4 kernel-column
    subshifts s=0..3 x 32 output channels.  A column block vb covers
    kernel columns 4*vb+s; the s-shift is resolved after accumulation by
    a shifted 4-way add across PSUM partition blocks.
  - Per output-row-pair: 8 row-groups x 8 col-blocks = 64 matmuls of
    [128 x 128] @ [128 x (2*229)] accumulated in one PSUM bank, then a
    3-op vector epilogue (2 shifted adds + fused add+bias) and a DMA out.

Kernel weights/columns beyond 31 are zero-padded on the host; the signal
is zero-padded by one row/column in SBUF so the padded taps multiply
zeros (never uninitialized memory).
"""

import os
import sys

import numpy as np

for _p in ("/opt/trn_rl_repo",):
    if _p not in sys.path and os.path.isdir(_p):
        sys.path.insert(0, _p)

import concourse.bacc as bacc
import concourse.mybir as mybir
import concourse.tile as tile
from concourse.bass_utils import run_bass_kernel_spmd

# Problem constants (hardcoded per harness contract).
B, C, H, W = 16, 32, 256, 256
D, KH = 32, 31
TH = TW = 226
NCORES = 8
BPC = B // NCORES  # batches per core
HALO = 30          # extra sigrep rows below a tile (28 group offset + 2 wrap)
# Output-row tiles (start, nrows); nrows even.
ROW_TILES = [(0, 38), (38, 38), (76, 38), (114, 38), (152, 38), (190, 36)]

_DT_NAMES = {
    "f32r": mybir.dt.float32r,
    "f32": mybir.dt.float32,
    "f16": mybir.dt.float16,
    "bf16": mybir.dt.bfloat16,
}
DT_KEY = os.environ.get("FFTCONV_DT", "f32r")


def _np_dt(dt_mm):
    return mybir.dt.np(dt_mm)


def build_program(dt_key: str = DT_KEY):
    """Build the SPMD Bass program (one NeuronCore's slice: BPC batches)."""
    dt_mm = _DT_NAMES[dt_key]
    f32 = mybir.dt.float32
    nc = bacc.Bacc(
        "TRN2",
        target_bir_lowering=False,
        debug=False,
        enable_asserts=False,
        num_devices=NCORES,
    )
    sig_d = nc.dram_tensor("signal", [BPC, C, H, W], dt_mm, kind="ExternalInput")
    wt_d = nc.dram_tensor("wT", [128, 8, 8, 128], dt_mm, kind="ExternalInput")
    bias_d = nc.dram_tensor("bias", [D, 1], f32, kind="ExternalInput")
    out_d = nc.dram_tensor("out", [BPC, D, TH, TW], f32, kind="ExternalOutput")

    with tile.TileContext(nc) as tc:
        with (
            tc.tile_pool(name="const", bufs=1) as const_pool,
            tc.tile_pool(name="sig", bufs=2) as sig_pool,
            tc.tile_pool(name="psum", bufs=8, space="PSUM") as psum_pool,
            tc.tile_pool(name="tmp", bufs=2) as tmp_pool,
            tc.tile_pool(name="outb", bufs=4) as out_pool,
        ):
            wt = const_pool.tile([128, 8, 8, 128], dt_mm)
            nc.sync.dma_start(wt[:, :, :, :], wt_d[:, :, :, :])
            bias_t = const_pool.tile([D, 1], f32)
            nc.sync.dma_start(bias_t[:, :], bias_d[:, :])

            for b in range(BPC):
                for i0, R in ROW_TILES:
                    # Signal rows stored CONTIGUOUSLY at pitch 256 (= W) so a
                    # two-row matmul rhs is one flat 512 span (float32r
                    # requires a 2-D moving AP).  Column overruns wrap to the
                    # next row but only land on zero-weight taps / unread
                    # psum columns.
                    r_tot = R + HALO
                    srep = sig_pool.tile([128, r_tot * W], dt_mm, tag="srep")
                    srep3 = srep[:].rearrange("p (r w) -> p r w", w=W)
                    for u in range(4):
                        rows = min(r_tot, H - (i0 + u))
                        nc.sync.dma_start(
                            srep3[u * 32 : (u + 1) * 32, 0:rows, :],
                            sig_d[b, :, i0 + u : i0 + u + rows, :],
                        )
                        if rows < r_tot:
                            nc.vector.memset(
                                srep3[u * 32 : (u + 1) * 32, rows:r_tot, :].bitcast(
                                    mybir.dt.float32
                                ),
                                0.0,
                            )
                    for rp in range(R // 2):
                        r2 = 2 * rp
                        i = i0 + r2
                        ps = psum_pool.tile([128, 2 * W], f32, tag="ps")
                        ps3 = ps[:].rearrange("p (r w) -> p r w", w=W)
                        for g in range(8):
                            for vb in range(8):
                                off = (r2 + 4 * g) * W + 4 * vb
                                nc.tensor.matmul(
                                    ps[:, :],
                                    lhsT=wt[:, g, vb, :],
                                    rhs=srep[:, off : off + 2 * W],
                                    start=(g == 0 and vb == 0),
                                    stop=(g == 7 and vb == 7),
                                )
                        # One PSUM operand per instruction (HW: single DVE
                        # PSUM read port).  ACT folds in the bias.
                        t0 = tmp_pool.tile([D, 2, TW], f32, tag="t0")
                        t1 = tmp_pool.tile([D, 2, TW], f32, tag="t1")
                        t2 = tmp_pool.tile([D, 2, TW], f32, tag="t2")
                        ob = out_pool.tile([D, 2, TW], f32, tag="ob")
                        nc.scalar.activation(
                            t0[:, :, :],
                            ps3[0:32, :, 0:226],
                            mybir.ActivationFunctionType.Identity,
                            bias=bias_t[:, :],
                        )
                        nc.vector.tensor_add(t1[:, :, :], t0[:, :, :], ps3[32:64, :, 1:227])
                        nc.vector.tensor_add(t2[:, :, :], t1[:, :, :], ps3[64:96, :, 2:228])
                        nc.vector.tensor_add(ob[:, :, :], t2[:, :, :], ps3[96:128, :, 3:229])
                        nc.sync.dma_start(out_d[b, :, i : i + 2, :], ob[:, :, :])
    nc.compile()
    return nc


def pack_weights(weight: np.ndarray, np_dt) -> np.ndarray:
    """weight [D, C, 31, 31] -> lhsT table [128, 8, 8, 128].

    wT[(u_idx*32 + c), g, vb, (s*32 + d)] = weight[d, c, 4g+u_idx, 4vb+s],
    zero where 4g+u_idx > 30 or 4vb+s > 30.
    """
    w = np.zeros((D, C, 32, 32), np.float32)
    w[:, :, :KH, :KH] = weight.astype(np.float32)
    # -> [u_idx, c, g, vb, s, d]
    wt = w.reshape(D, C, 8, 4, 8, 4).transpose(3, 1, 2, 4, 5, 0)
    wt = wt.reshape(4 * C, 8, 8, 4 * D)
    return np.ascontiguousarray(wt.astype(np_dt))


_PROGRAM_CACHE: dict[str, object] = {}


def _get_program(dt_key: str):
    prog = _PROGRAM_CACHE.get(dt_key)
    if prog is None:
        prog = build_program(dt_key)
        _PROGRAM_CACHE[dt_key] = prog
    return prog


def make_in_maps(signal, weight, bias, dt_key: str = DT_KEY):
    np_dt = _np_dt(_DT_NAMES[dt_key])
    wT = pack_weights(np.asarray(weight), np_dt)
    sig = np.asarray(signal).astype(np_dt, copy=False)
    b2 = np.ascontiguousarray(np.asarray(bias, np.float32).reshape(D, 1))
    return [
        {
            "signal": np.ascontiguousarray(sig[c * BPC : (c + 1) * BPC]),
            "wT": wT,
            "bias": b2,
        }
        for c in range(NCORES)
    ]


def kernel(signal, weight, bias):
    nc = _get_program(DT_KEY)
    in_maps = make_in_maps(signal, weight, bias, DT_KEY)
    res = run_bass_kernel_spmd(nc, in_maps, list(range(NCORES)))
    out = np.concatenate([res.results[c]["out"] for c in range(NCORES)], axis=0)
    return np.ascontiguousarray(out.astype(np.float32, copy=False))


# revision 13
# speedup vs baseline: 1.1632x; 1.1632x over previous
"""FFT-Conv2d as direct valid cross-correlation on Trainium2 (Bass/Tile).

Math: the reference's rfft2/einsum/irfft2 pipeline is exactly a *valid*
2-D cross-correlation plus bias:

    out[b, d, i, j] = sum_{c,u,v} signal[b, c, i+u, j+v] * weight[d, c, u, v]
                      + bias[d]

with signal [16, 32, 256, 256], weight [32, 32, 31, 31] -> out [16, 32, 226, 226].

Device strategy (data-parallel, 2 batch images per NeuronCore x 8 cores):
  - Contraction dim (PE partition axis, K=128) packs 4 kernel rows x 32
    input channels.  The signal is replicated into SBUF 4x with row shifts
    of 0..3 so that one SBUF row slice provides all 128 contraction rows.
  - Output dim (PE partition axis of PSUM, M=128) packs 4 kernel-column
    subshifts s=0..3 x 32 output channels.  A column block vb covers
    kernel columns 4*vb+s; the s-shift is resolved after accumulation by
    a shifted 4-way add across PSUM partition blocks.
  - Per output-row-pair: 8 row-groups x 8 col-blocks = 64 matmuls of
    [128 x 128] @ [128 x (2*229)] accumulated in one PSUM bank, then a
    3-op vector epilogue (2 shifted adds + fused add+bias) and a DMA out.

Kernel weights/columns beyond 31 are zero-padded on the host; the signal
is zero-padded by one row/column in SBUF so the padded taps multiply
zeros (never uninitialized memory).
"""

import os
import sys

import numpy as np

for _p in ("/opt/trn_rl_repo",):
    if _p not in sys.path and os.path.isdir(_p):
        sys.path.insert(0, _p)

import concourse.bacc as bacc
import concourse.mybir as mybir
import concourse.tile as tile
from concourse.bass_utils import run_bass_kernel_spmd

# Problem constants (hardcoded per harness contract).
B, C, H, W = 16, 32, 256, 256
D, KH = 32, 31
TH = TW = 226
NCORES = 8
BPC = B // NCORES  # batches per core
HALO = 30          # extra sigrep rows below a tile (28 group offset + 2 wrap)
# Output-row tiles (start, nrows); nrows even.
ROW_TILES = [(0, 38), (38, 38), (76, 38), (114, 38), (152, 38), (190, 36)]

# key -> (weight dtype, signal dtype, use 3-D two-row rhs AP of width 229)
# float32r requires a 2-D (flat 512) moving AP; 16-bit dtypes can use the
# narrower 3-D AP (458 streamed columns instead of 512).
_DT_CONFIGS = {
    "f32r": (mybir.dt.float32r, mybir.dt.float32r, False),
    "f16": (mybir.dt.float16, mybir.dt.float16, True),
    "f16flat": (mybir.dt.float16, mybir.dt.float16, False),
    "bf16": (mybir.dt.bfloat16, mybir.dt.bfloat16, True),
    "f32": (mybir.dt.float32, mybir.dt.float32, True),
}
DT_KEY = os.environ.get("FFTCONV_DT", "f32r")


def _np_dt(dt_mm):
    return mybir.dt.np(dt_mm)


def build_program(dt_key: str = DT_KEY):
    """Build the SPMD Bass program (one NeuronCore's slice: BPC batches)."""
    wt_dt, dt_mm, use3d = _DT_CONFIGS[dt_key]
    f32 = mybir.dt.float32
    NJ = 229
    nc = bacc.Bacc(
        "TRN2",
        target_bir_lowering=False,
        debug=False,
        enable_asserts=False,
        num_devices=NCORES,
    )
    sig_d = nc.dram_tensor("signal", [BPC, C, H, W], dt_mm, kind="ExternalInput")
    wt_d = nc.dram_tensor("wT", [128, 8, 8, 128], wt_dt, kind="ExternalInput")
    bias_d = nc.dram_tensor("bias", [D, 1], f32, kind="ExternalInput")
    out_d = nc.dram_tensor("out", [BPC, D, TH, TW], f32, kind="ExternalOutput")

    with tile.TileContext(nc) as tc:
        with (
            tc.tile_pool(name="const", bufs=1) as const_pool,
            tc.tile_pool(name="sig", bufs=2) as sig_pool,
            tc.tile_pool(name="psum", bufs=8, space="PSUM") as psum_pool,
            tc.tile_pool(name="tmp", bufs=2) as tmp_pool,
            tc.tile_pool(name="outb", bufs=4) as out_pool,
        ):
            wt = const_pool.tile([128, 8, 8, 128], wt_dt)
            nc.sync.dma_start(wt[:, :, :, :], wt_d[:, :, :, :])
            bias_t = const_pool.tile([D, 1], f32)
            nc.sync.dma_start(bias_t[:, :], bias_d[:, :])

            for b in range(BPC):
                for i0, R in ROW_TILES:
                    # Signal rows stored CONTIGUOUSLY at pitch 256 (= W) so a
                    # two-row matmul rhs is one flat 512 span (float32r
                    # requires a 2-D moving AP).  Column overruns wrap to the
                    # next row but only land on zero-weight taps / unread
                    # psum columns.
                    r_tot = R + HALO
                    srep = sig_pool.tile([128, r_tot * W], dt_mm, tag="srep")
                    srep3 = srep[:].rearrange("p (r w) -> p r w", w=W)
                    for u in range(4):
                        rows = min(r_tot, H - (i0 + u))
                        nc.sync.dma_start(
                            srep3[u * 32 : (u + 1) * 32, 0:rows, :],
                            sig_d[b, :, i0 + u : i0 + u + rows, :],
                        )
                        if rows < r_tot:
                            nc.vector.memset(
                                srep3[u * 32 : (u + 1) * 32, rows:r_tot, :].bitcast(
                                    mybir.dt.float32
                                ),
                                0.0,
                            )
                    for rp in range(R // 2):
                        r2 = 2 * rp
                        i = i0 + r2
                        if use3d:
                            ps3 = psum_pool.tile([128, 2, NJ], f32, tag="ps")
                        else:
                            ps = psum_pool.tile([128, 2 * W], f32, tag="ps")
                            ps3 = ps[:].rearrange("p (r w) -> p r w", w=W)
                        for g in range(8):
                            for vb in range(8):
                                off = (r2 + 4 * g) * W + 4 * vb
                                if use3d:
                                    rhs = srep[:, off : off + 2 * W].rearrange(
                                        "p (r w) -> p r w", w=W
                                    )[:, :, 0:NJ]
                                    nc.tensor.matmul(
                                        ps3[:, :, :],
                                        lhsT=wt[:, g, vb, :],
                                        rhs=rhs,
                                        start=(g == 0 and vb == 0),
                                        stop=(g == 7 and vb == 7),
                                    )
                                else:
                                    nc.tensor.matmul(
                                        ps[:, :],
                                        lhsT=wt[:, g, vb, :],
                                        rhs=srep[:, off : off + 2 * W],
                                        start=(g == 0 and vb == 0),
                                        stop=(g == 7 and vb == 7),
                                    )
                        # One PSUM operand per instruction (HW: single DVE
                        # PSUM read port).  ACT folds in the bias.
                        t0 = tmp_pool.tile([D, 2, TW], f32, tag="t0")
                        t1 = tmp_pool.tile([D, 2, TW], f32, tag="t1")
                        t2 = tmp_pool.tile([D, 2, TW], f32, tag="t2")
                        ob = out_pool.tile([D, 2, TW], f32, tag="ob")
                        nc.scalar.activation(
                            t0[:, :, :],
                            ps3[0:32, :, 0:226],
                            mybir.ActivationFunctionType.Identity,
                            bias=bias_t[:, :],
                        )
                        nc.vector.tensor_add(t1[:, :, :], t0[:, :, :], ps3[32:64, :, 1:227])
                        nc.vector.tensor_add(t2[:, :, :], t1[:, :, :], ps3[64:96, :, 2:228])
                        nc.vector.tensor_add(ob[:, :, :], t2[:, :, :], ps3[96:128, :, 3:229])
                        nc.sync.dma_start(out_d[b, :, i : i + 2, :], ob[:, :, :])
    nc.compile()
    return nc


def pack_weights(weight: np.ndarray, np_dt) -> np.ndarray:
    """weight [D, C, 31, 31] -> lhsT table [128, 8, 8, 128].

    wT[(u_idx*32 + c), g, vb, (s*32 + d)] = weight[d, c, 4g+u_idx, 4vb+s],
    zero where 4g+u_idx > 30 or 4vb+s > 30.
    """
    w = np.zeros((D, C, 32, 32), np.float32)
    w[:, :, :KH, :KH] = weight.astype(np.float32)
    # -> [u_idx, c, g, vb, s, d]
    wt = w.reshape(D, C, 8, 4, 8, 4).transpose(3, 1, 2, 4, 5, 0)
    wt = wt.reshape(4 * C, 8, 8, 4 * D)
    return np.ascontiguousarray(wt.astype(np_dt))


_PROGRAM_CACHE: dict[str, object] = {}


def _get_program(dt_key: str):
    prog = _PROGRAM_CACHE.get(dt_key)
    if prog is None:
        prog = build_program(dt_key)
        _PROGRAM_CACHE[dt_key] = prog
    return prog


def make_in_maps(signal, weight, bias, dt_key: str = DT_KEY):
    wt_dt, sig_dt, _ = _DT_CONFIGS[dt_key]
    wT = pack_weights(np.asarray(weight), _np_dt(wt_dt))
    sig = np.asarray(signal).astype(_np_dt(sig_dt), copy=False)
    b2 = np.ascontiguousarray(np.asarray(bias, np.float32).reshape(D, 1))
    return [
        {
            "signal": np.ascontiguousarray(sig[c * BPC : (c + 1) * BPC]),
            "wT": wT,
            "bias": b2,
        }
        for c in range(NCORES)
    ]


def kernel(signal, weight, bias):
    nc = _get_program(DT_KEY)
    in_maps = make_in_maps(signal, weight, bias, DT_KEY)
    res = run_bass_kernel_spmd(nc, in_maps, list(range(NCORES)))
    out = np.concatenate([res.results[c]["out"] for c in range(NCORES)], axis=0)
    return np.ascontiguousarray(out.astype(np.float32, copy=False))
